# revision 45
# baseline (speedup 1.0000x reference)
"""Multi-head attention (B=2, S=2048, D=768, H=12) on 8 TRN2 NeuronCores.

Sharding: core c -> batch b = c//4, head-group g = c%4 (3 heads of 64 each).

v2.2 design (vs v1 baseline ~285us):
  - Fused attention pipeline per 2-ktile group: energy MMs -> exp (ACT) ->
    mask mul (DVE) -> PV accumulation (software-pipelined, PV lags 2 groups)
    so the scalar engine (exp, ~110us total) paces and the rest hides.
  - Energy head-2 alternates PE row-groups by group parity (duplicated
    qB2/kB2 halves) so 2 of 3 energy matmuls always run concurrently.
  - PV via ones-augmented v (denominator for free, [65,512] PSUM accums).
  - Block-boundary decoupling: ou PSUM staged to SBUF immediately (PV of
    next block reuses the bank after ~2us instead of waiting the whole
    reciprocal/broadcast/normalize chain); denominator rows gathered by DMA
    into one [3,512] tile -> a single batched reciprocal per block.
  - k-outer weight-stationary projections, psB n-pairs col-tiled into one
    PSUM bank; order q -> v (+PE transposes) -> k chasing x DMA arrivals.
  - Out-projection packed K=128(h0+h1)+K=64(h2), interleaved into the next
    block, 2-piece PSUM ([128,512]+[128,256] in a dedicated bank).
  - DMA dispatch: bulk (x, mask, out, partition-moves) on gpsimd swDGE
    (16-queue spread), weights on sync hwDGE; proj copies on the
    ramp-idle scalar engine; o_sb copies on gpsimd.
"""

import os
import sys

sys.path.insert(0, "/opt/trn_rl_repo")

from contextlib import ExitStack

import ml_dtypes
import numpy as np

import concourse.bass as bass
import concourse.mybir as mybir
import concourse.tile as tile
from concourse import bacc
from concourse.bass import ds
from concourse.bass_utils import run_bass_kernel_spmd
from concourse.masks import make_identity

F32 = mybir.dt.float32
BF16 = mybir.dt.bfloat16

SEQ = 2048
D = 768
HD = 64
GD = 192
QB = 512
NQB = SEQ // QB   # 4
KT = SEQ // 128   # 16
NG = 8            # 2-ktile groups per block
SCALE = float(1.0 / np.sqrt(np.float32(D)))
PV_LAG = 2

DEBUG = bool(int(os.environ.get("KERNEL_DEBUG", "0")))

_CACHE = {}


def _install_profile_hook():
    import types

    if "antenv.axon_hooks" in sys.modules:
        return
    sys.path.insert(0, "/root/.axon_site")
    try:
        from trn_agent_boot.trn_boot import _ntff_profile_via_ctypes
        hook = _ntff_profile_via_ctypes("/opt/axon/libaxon_pjrt.so")
    except Exception:
        hook = None
    import concourse.bass_utils as _bu

    _bu.upload_artifacts = lambda tmpdir: tmpdir
    mod = types.ModuleType("antenv.axon_hooks")
    mod.get_axon_ntff_profile_hook = lambda: hook
    mod.set_axon_ntff_profile_hook = lambda h: None
    sys.modules["antenv.axon_hooks"] = mod


def _build():
    nc = bacc.Bacc(None)
    Exp = mybir.ActivationFunctionType.Exp

    xq = nc.declare_dram_parameter("xq", [6, 128, SEQ], BF16, isOutput=False)
    xk = nc.declare_dram_parameter("xk", [6, 128, SEQ], BF16, isOutput=False)
    xv = nc.declare_dram_parameter("xv", [6, 128, SEQ], BF16, isOutput=False)
    wq = nc.declare_dram_parameter("wq", [128, 6, GD], BF16, isOutput=False)
    wk = nc.declare_dram_parameter("wk", [128, 6, GD], BF16, isOutput=False)
    wv = nc.declare_dram_parameter("wv", [128, 6, GD], BF16, isOutput=False)
    woT = nc.declare_dram_parameter("woT", [GD, D], BF16, isOutput=False)
    maskT = nc.declare_dram_parameter("maskT", [SEQ, SEQ], BF16, isOutput=False)
    out = nc.declare_dram_parameter("out", [SEQ, D], F32, isOutput=True)
    if DEBUG:
        dbg_qA = nc.declare_dram_parameter("dbg_qA", [128, SEQ], BF16, isOutput=True)
        dbg_kA = nc.declare_dram_parameter("dbg_kA", [128, SEQ], BF16, isOutput=True)
        dbg_va = nc.declare_dram_parameter("dbg_va", [128, KT * 65], BF16, isOutput=True)
        dbg_P = nc.declare_dram_parameter("dbg_P", [128, KT * QB], BF16, isOutput=True)
        dbg_onA = nc.declare_dram_parameter("dbg_onA", [128, SEQ], BF16, isOutput=True)
        dbg_onB = nc.declare_dram_parameter("dbg_onB", [64, SEQ], BF16, isOutput=True)

    with tile.TileContext(nc) as tc, ExitStack() as ctx:
        cpool = ctx.enter_context(tc.tile_pool(name="const", bufs=1))
        ident = cpool.tile([128, 128], BF16)
        make_identity(nc, ident[:])

        # ---- persistent SBUF -----------------------------------------------
        pp = ctx.enter_context(tc.tile_pool(name="persist", bufs=1))
        qA = pp.tile([128, SEQ], BF16, tag="qA")    # h0 p0-63, h1 p64-127
        kA = pp.tile([128, SEQ], BF16, tag="kA")
        qB2 = pp.tile([128, SEQ], BF16, tag="qB2")  # h2 duplicated both halves
        kB2 = pp.tile([128, SEQ], BF16, tag="kB2")
        vaug = [pp.tile([128, KT, HD + 1], BF16, tag=f"vaug{h}", name=f"vaug{h}")
                for h in range(3)]
        onormA = pp.tile([128, SEQ], BF16, tag="onA")
        onormB = pp.tile([64, SEQ], BF16, tag="onB")
        woA_sb = pp.tile([128, D], BF16, tag="woA")
        woB_sb = pp.tile([64, D], BF16, tag="woB")
        ones65 = pp.tile([65, HD], F32, tag="ones65")
        ones65b = pp.tile([65, HD], BF16, tag="ones65b")

        w_sb = {}
        for name, wT in (("q", wq), ("k", wk), ("v", wv)):
            w_sb[name] = pp.tile([128, 6, GD], BF16, tag=f"w{name}", name=f"w_{name}")
            nc.sync.dma_start(w_sb[name][:], wT[:, :, :])
        nc.sync.dma_start(woA_sb[:], woT[0:128, :])
        nc.sync.dma_start(woB_sb[:], woT[128:GD, :])

        for h in range(3):
            nc.vector.memset(vaug[h][:, :, HD:HD + 1], 1.0)
        nc.vector.memset(ones65[:], 1.0)
        nc.vector.memset(ones65b[:], 1.0)

        mp = ctx.enter_context(tc.tile_pool(name="mp", bufs=2))
        maskR = maskT.rearrange("(j p) q -> p j q", p=128)
        mask_t = {}

        def issue_mask(n):
            mask_t[n] = mp.tile([128, KT, QB], BF16, tag="mask", name=f"mask{n}")
            for j in range(0, KT, 2):
                nc.gpsimd.dma_start(
                    mask_t[n][:, ds(j, 2), :], maskR[:, ds(j, 2), ds(n * QB, QB)]
                )

        dummy = cpool.tile([1, 2], F32)
        nc.scalar.activation(dummy[:], ident[0:1, 0:2], Exp, scale=1.0)

        # ---- phase A: projections ------------------------------------------
        with tc.tile_pool(name="xs", bufs=1) as xs, \
             tc.tile_pool(name="vtp", bufs=1) as vtp, \
             tc.tile_pool(name="pjA", bufs=1, space="PSUM") as pjA, \
             tc.tile_pool(name="pjB", bufs=1, space="PSUM") as pjB, \
             tc.tile_pool(name="tr_ps", bufs=1, space="PSUM") as tr_ps:
            vtA = vtp.tile([128, SEQ], BF16, tag="vtA")
            vtB = vtp.tile([64, SEQ], BF16, tag="vtB")
            x_sb = {}
            for name in ("q", "v", "k"):
                x_sb[name] = [
                    xs.tile([128, SEQ], BF16, tag=f"x{name}{k}",
                            name=f"x_{name}{k}")
                    for k in range(6)
                ]
            # all x dispatches first (q, v, k order = consumption order)
            xsrc = {"q": xq, "v": xv, "k": xk}
            for name in ("q", "v", "k"):
                for k in range(6):
                    for half in range(2):
                        nc.gpsimd.dma_start(
                            x_sb[name][k][:, ds(half * 1024, 1024)],
                            xsrc[name][ds(k, 1), :, ds(half * 1024, 1024)],
                        )
            def proj(name, dA, dB2):
                """k-outer weight-stationary projection.
                dA: [128,SEQ] dest (M=128 half); dB2: [128,SEQ] dest for the
                M=64 half, n-pairs col-tiled (even n -> p0-63, odd -> p64-127).
                """
                psA = [pjA.tile([128, QB], F32, tag=f"pA{n}", name=f"pA_{name}{n}")
                       for n in range(NQB)]
                psB = [pjB.tile([128, QB], F32, tag=f"pB{p}", name=f"pB_{name}{p}")
                       for p in range(NQB // 2)]
                for k in range(6):
                    for n in range(NQB):
                        nc.tensor.matmul(
                            psA[n][:],
                            lhsT=w_sb[name][:, k, 0:128],
                            rhs=x_sb[name][k][:, ds(n * QB, QB)],
                            start=(k == 0), stop=(k == 5),
                        )
                    for p in range(NQB // 2):
                        for half in range(2):
                            n = 2 * p + half
                            nc.tensor.matmul(
                                psB[p][ds(half * 64, 64), :],
                                lhsT=w_sb[name][:, k, 128:GD],
                                rhs=x_sb[name][k][:, ds(n * QB, QB)],
                                start=(k == 0), stop=(k == 5),
                                tile_position=(0, half * 64),
                            )
                for n in range(NQB):
                    nc.scalar.copy(dA[:, ds(n * QB, QB)], psA[n][:])
                for p in range(NQB // 2):
                    for half in range(2):
                        n = 2 * p + half
                        b0 = half * 64
                        nc.scalar.copy(
                            dB2[b0:b0 + 64, ds(n * QB, QB)],
                            psB[p][b0:b0 + 64, :])
                # duplicate the B half into the other row-group via DMA
                for n in range(NQB):
                    src = (n % 2) * 64
                    dst = 64 - src
                    nc.gpsimd.dma_start(
                        dB2[dst:dst + 64, ds(n * QB, QB)],
                        dB2[src:src + 64, ds(n * QB, QB)])

            proj("q", qA, qB2)
            # v projection into vt staging, then PE transposes
            vB2 = vtp.tile([128, SEQ], BF16, tag="vB2")
            proj("v", vtA, vB2)
            # vtB: take each n's valid half of vB2 (even n p0-63, odd p64-127)
            for n in range(NQB):
                src = (n % 2) * 64
                nc.gpsimd.dma_start(
                    vtB[:, ds(n * QB, QB)], vB2[src:src + 64, ds(n * QB, QB)])
            for s in range(KT):
                ptA = tr_ps.tile([128, 128], BF16, tag="ptA")
                nc.tensor.transpose(ptA[:], vtA[:, ds(s * 128, 128)], ident[:])
                nc.vector.tensor_copy(vaug[0][:, s, 0:HD], ptA[:, 0:64])
                nc.vector.tensor_copy(vaug[1][:, s, 0:HD], ptA[:, 64:128])
                ptB = tr_ps.tile([128, 64], BF16, tag="ptB")
                nc.tensor.transpose(
                    ptB[:], vtB[0:64, ds(s * 128, 128)], ident[0:64, 0:64])
                nc.vector.tensor_copy(vaug[2][:, s, 0:HD], ptB[:, 0:64])
            proj("k", kA, kB2)
            issue_mask(0)
            issue_mask(1)

        # ---- phase B: fused attention + out-projection ---------------------
        pp2 = ctx.enter_context(tc.tile_pool(name="pp2", bufs=2))
        rp = ctx.enter_context(tc.tile_pool(name="rp", bufs=2))
        op = ctx.enter_context(tc.tile_pool(name="op", bufs=3))
        sg = ctx.enter_context(tc.tile_pool(name="sg", bufs=1))

        with tc.tile_pool(name="e_ps", bufs=2, space="PSUM") as e_ps, \
             tc.tile_pool(name="ou_ps", bufs=1, space="PSUM") as ou_ps, \
             tc.tile_pool(name="f_ps", bufs=1, space="PSUM") as f_ps:

            P = {}
            ou = {}
            ou_sb = {}

            def e_mms(n, g, h):
                e = e_ps.tile([128, 2, QB], F32, tag="e", name=f"e{n}_{g}_{h}")
                if h == 0:
                    src_q, src_k, p0 = qA, kA, 0
                elif h == 1:
                    src_q, src_k, p0 = qA, kA, 64
                else:
                    # h2 alternates row groups by m parity
                    src_q, src_k, p0 = qB2, kB2, 0
                for mm in range(2):
                    m = 2 * g + mm
                    if h == 2:
                        p0 = (0 if (m % 2 == 0) else 64)
                    nc.tensor.matmul(
                        e[:, mm, :],
                        lhsT=src_k[p0:p0 + 64, ds(m * 128, 128)],
                        rhs=src_q[p0:p0 + 64, ds(n * QB, QB)],
                        start=True, stop=True,
                    )
                return e

            def exp_op(n, g, h, e):
                nc.scalar.activation(
                    P[n, h][:, ds(2 * g, 2), :], e[:, :, :], Exp, scale=SCALE)

            def mul_op(n, gg, h):
                sl = ds(4 * gg, 4)
                nc.vector.tensor_mul(
                    P[n, h][:, sl, :], P[n, h][:, sl, :], mask_t[n][:, sl, :])

            def pv_mms(n, g, h):
                for mm in range(2):
                    m = 2 * g + mm
                    nc.tensor.matmul(
                        ou[n, h][:],
                        lhsT=vaug[h][:, m, :],
                        rhs=P[n, h][:, m, :],
                        start=(m == 0), stop=(m == KT - 1),
                    )

            def stage_head(n, h):
                """right after head h's last PV: stage its accumulator to
                SBUF and park its denominator row at psum partition 32h."""
                if h == 0:
                    r3cur[n] = f_ps.tile([65, QB], F32, tag="f", name=f"r3_{n}")
                ou_sb[n, h] = sg.tile(
                    [HD + 1, QB], F32, tag=f"os{h}", name=f"os{n}_{h}")
                nc.vector.tensor_copy(ou_sb[n, h][:], ou[n, h][:])
                nc.tensor.matmul(
                    r3cur[n][32 * h:32 * h + 1, :],
                    lhsT=ones65[64:65, 0:1],
                    rhs=ou_sb[n, h][HD:HD + 1, :],
                    start=True, stop=True)

            def stage_fin(n):
                ri = rp.tile([65, QB], BF16, tag="ri", name=f"ri_{n}")
                with nc.allow_low_precision(reason="1/denominator in bf16"):
                    nc.vector.reciprocal(ri[:], r3cur[n][:])
                return ri

            r3cur = {}

            def norm_op(n, h, ri):
                nsl = ds(n * QB, QB)
                rbps = f_ps.tile([64, QB], F32, tag="f", name=f"rb_{n}_{h}")
                nc.tensor.matmul(
                    rbps[:],
                    lhsT=ones65b[32 * h:32 * h + 1, 0:64],
                    rhs=ri[32 * h:32 * h + 1, :],
                    start=True, stop=True)
                if h == 0:
                    nc.vector.tensor_mul(
                        onormA[0:64, nsl], ou_sb[n, h][0:HD, :], rbps[:])
                elif h == 1:
                    tmp1 = rp.tile([64, QB], BF16, tag="t1", name=f"t1_{n}")
                    nc.vector.tensor_mul(tmp1[:], ou_sb[n, h][0:HD, :], rbps[:])
                    nc.gpsimd.dma_start(onormA[64:128, nsl], tmp1[:])
                else:
                    nc.vector.tensor_mul(
                        onormB[:, nsl], ou_sb[n, h][0:HD, :], rbps[:])

            def outproj(n, mqi, split_dma=False):
                mq = 4 * n + mqi
                msl = ds(mq * 128, 128)
                o_sb = op.tile([128, D], F32, tag="o", name=f"o{mq}")
                for n0, nw in ((0, 512), (512, 256)):
                    if split_dma and n0 == 512:
                        fpt = e_ps.tile(
                            [128, 2, QB], F32, tag="e", name=f"fp{mq}_{n0}"
                        )[:].rearrange("p a b -> p (a b)")[:, 0:QB]
                    else:
                        fpt = f_ps.tile(
                            [128, QB], F32, tag="f", name=f"fp{mq}_{n0}")
                    nc.tensor.matmul(
                        fpt[:, 0:nw], lhsT=onormA[:, msl],
                        rhs=woA_sb[:, ds(n0, nw)], start=True, stop=False)
                    nc.tensor.matmul(
                        fpt[:, 0:nw], lhsT=onormB[:, msl],
                        rhs=woB_sb[:, ds(n0, nw)], start=False, stop=True)
                    nc.vector.tensor_copy(o_sb[:, ds(n0, nw)], fpt[:, 0:nw])
                if split_dma:
                    for c0 in (0, 256, 512):
                        nc.gpsimd.dma_start(
                            out[msl, ds(c0, 256)], o_sb[:, ds(c0, 256)])
                else:
                    nc.gpsimd.dma_start(out[msl, :], o_sb[:])

            ri_prev = None
            for n in range(NQB):
                if 1 <= n < NQB - 1:
                    issue_mask(n + 1)
                for h in range(3):
                    P[n, h] = pp2.tile(
                        [128, KT, QB], BF16, tag=f"P{h}", name=f"P{n}_{h}")
                    ou[n, h] = ou_ps.tile(
                        [HD + 1, QB], F32, tag=f"ou{h}", name=f"ou{n}_{h}")

                for g in range(NG):
                    for h in range(3):
                        e = e_mms(n, g, h)
                        exp_op(n, g, h, e)
                    if g % 2 == 1:
                        for h in range(3):
                            mul_op(n, g // 2, h)
                    gl = g - PV_LAG
                    if gl >= 0:
                        for h in range(3):
                            pv_mms(n, gl, h)
                    # previous block's normalization + out-projection
                    if n >= 1:
                        if g == 2:
                            for h in range(3):
                                norm_op(n - 1, h, ri_prev)
                        elif 3 <= g < 7:
                            outproj(n - 1, g - 3)
                for gl in range(NG - PV_LAG, NG):
                    for h in range(3):
                        pv_mms(n, gl, h)
                for h in range(3):
                    stage_head(n, h)
                ri_prev = stage_fin(n)

            for h in range(3):
                norm_op(NQB - 1, h, ri_prev)
            for mqi in range(4):
                outproj(NQB - 1, mqi, split_dma=True)


            if DEBUG:
                nc.gpsimd.dma_start(dbg_qA[:, :], qA[:])
                nc.gpsimd.dma_start(dbg_kA[:, :], kA[:])
                nc.gpsimd.dma_start(
                    dbg_va[:, :], vaug[0][:].rearrange("p a b -> p (a b)"))
                nc.gpsimd.dma_start(
                    dbg_P[:, :],
                    P[NQB - 1, 0][:].rearrange("p a b -> p (a b)"))
                nc.gpsimd.dma_start(dbg_onA[:, :], onormA[:])
                nc.gpsimd.dma_start(dbg_onB[:, :], onormB[:])

    nc.compile()
    return nc


def kernel(Q, K, V, mask, Wq, Wk, Wv, Wo):
    if "nc" not in _CACHE:
        _CACHE["nc"] = _build()
    nc = _CACHE["nc"]

    maskT_bf = np.ascontiguousarray(
        (mask[0, 0].T != 0).astype(ml_dtypes.bfloat16)
    )
    in_maps = []
    for c in range(8):
        b, g = c // 4, c % 4
        sl = slice(g * GD, (g + 1) * GD)

        def prep_x(X):
            return np.ascontiguousarray(
                X[b].T.reshape(6, 128, SEQ).astype(ml_dtypes.bfloat16))

        def prep_w(W):
            wT = W[sl, :].T.reshape(6, 128, GD)
            return np.ascontiguousarray(
                wT.transpose(1, 0, 2).astype(ml_dtypes.bfloat16))

        in_maps.append(
            {
                "xq": prep_x(Q), "xk": prep_x(K), "xv": prep_x(V),
                "wq": prep_w(Wq), "wk": prep_w(Wk), "wv": prep_w(Wv),
                "woT": np.ascontiguousarray(
                    Wo[:, sl].T.astype(ml_dtypes.bfloat16)),
                "maskT": maskT_bf,
            }
        )

    _install_profile_hook()
    res = run_bass_kernel_spmd(
        nc,
        in_maps,
        core_ids=list(range(8)),
        trace=bool(int(os.environ.get("KERNEL_PROFILE", "0"))),
    )
    _CACHE["last_exec_ns"] = res.exec_time_ns
    _CACHE["last_res"] = res

    outp = np.zeros((2, SEQ, D), dtype=np.float32)
    for c in range(8):
        outp[c // 4] += res.results[c]["out"]
    return outp
